# revision 1
# baseline (speedup 1.0000x reference)
"""Bahdanau-style attention kernel for 8 Trainium2 NeuronCores.

Reference computation (per full batch of 64):
    attn_1 = h @ W_dec.T                      # (b, 512)
    attn_2 = V @ W_enc.T                      # (b, s, 512)
    scores = tanh(attn_1[:,None,:] + attn_2) @ w_full   # (b, s)
    alpha  = softmax(scores, -1)
    out    = einsum('bs,bse->be', alpha, V)

Sharding: data-parallel over batch, 8 batches per core, weights replicated.
Compute dtype bf16 on the TensorEngine (fp32 accumulate in PSUM).
"""

import numpy as np

B_FULL = 64
N_CORES = 8
B = B_FULL // N_CORES  # 8 batches per core
SEQ = 2048
D = 512  # enc_dim == dec_dim == attn_dim
P = 128
NT = SEQ // P  # 16 s-tiles of 128
KE = D // P    # 4 contraction tiles
AT = D // P    # 4 attn-dim tiles
SC = SEQ // 512  # 4 s-chunks of 512

_CACHE = {}


def _split_waits(nc, maxw=1):
    """walrus in this container accepts only one sync-wait per instruction;
    move excess waits onto dedicated same-engine NOPs placed just before."""
    import concourse.mybir as mybir

    n = 0
    for f in nc.m.functions:
        for bb in f.blocks:
            new_list = []
            for inst in bb.instructions:
                si = getattr(inst, "sync_info", None)
                waits = list(si.on_wait) if si and si.on_wait else []
                if len(waits) > maxw:
                    keep = waits[-maxw:]
                    extra = waits[:-maxw]
                    for j in range(0, len(extra), maxw):
                        nop = mybir.InstNoOp(
                            name=f"{inst.name}-wsplit{j}",
                            engine=inst.engine,
                            bass_nofuse=True,
                            sync_info=mybir.SyncInfo(
                                on_wait=extra[j : j + maxw], on_update=[]
                            ),
                        )
                        nc.register_instruction(nop, overwrite=True)
                        new_list.append(nop)
                        n += 1
                    si.on_wait = keep
                new_list.append(inst)
            bb.instructions[:] = new_list
    return n


def _build(
    reps=1,
    skip_vt=False,
    ke_count=KE,
    skip_scores=False,
    loop_iters=None,
    vt_mode="xbar4",  # "pe": TensorE transposes; "xbar"/"xbar4": DMA transpose
    nb=B,  # number of batch iterations (timing attribution only)
    ctx_mode="dve",  # "pe": 16 matmuls; "dve": VectorE FMA chain + 1 reduce MM
    xbar_calls=4,  # how many DMA-transpose calls per batch (1, 4, or 16)
):
    # reps>1 repeats the whole per-batch pipeline inside one NEFF; used only
    # for benchmarking (wall-clock slope isolates per-rep device time from
    # the ~80ms axon dispatch overhead). skip_* / ke_count build timing-only
    # variants with stages removed (numerically wrong, structurally valid).
    import concourse.bass as bass
    import concourse.mybir as mybir
    import concourse.tile as tile
    from concourse.masks import make_identity

    f32 = mybir.dt.float32
    bf16 = mybir.dt.bfloat16
    Tanh = mybir.ActivationFunctionType.Tanh
    Exp = mybir.ActivationFunctionType.Exp
    X = mybir.AxisListType.X
    ADD = mybir.AluOpType.add

    nc = bass.Bass()
    h_d = nc.declare_dram_parameter("h", [B, D], f32, isOutput=False)
    v_d = nc.declare_dram_parameter("V", [B, SEQ, D], f32, isOutput=False)
    wdec_d = nc.declare_dram_parameter("W_dec", [D, D], f32, isOutput=False)
    wenc_d = nc.declare_dram_parameter("W_enc", [D, D], f32, isOutput=False)
    wf_d = nc.declare_dram_parameter("w_full", [D], f32, isOutput=False)
    out_d = nc.declare_dram_parameter("out", [B, D], f32, isOutput=True)

    with tile.TileContext(nc) as tc:
        with (
            tc.tile_pool(name="const", bufs=1) as const,
            tc.tile_pool(name="vpool", bufs=4) as vpool,
            tc.tile_pool(name="vtpool", bufs=3) as vtpool,
            tc.tile_pool(name="tanhpool", bufs=8) as tanhpool,
            tc.tile_pool(name="smpool", bufs=3) as smpool,
        ):
            ident_bf16 = const.tile([P, P], bf16)
            make_identity(nc, ident_bf16)

            # long-lived small tensors
            wencT = const.tile([P, KE, D], bf16)   # [e_p, ke, a]
            attn1T = const.tile([P, AT, B], f32)   # [a_p, at, b]
            wfull_sb = const.tile([P, AT], bf16)   # [a_p, at]
            ones_f32 = const.tile([P, 1], f32)
            nc.vector.memset(ones_f32, 1.0)

            # ---------------- setup (weights, attn1) ----------------
            with (
                tc.tile_pool(name="setup_sb", bufs=1) as setup_sb,
                tc.tile_pool(name="setup_ps", bufs=2, space="PSUM") as setup_ps,
            ):
                # h -> hT  [d_p, kd, b]  (bf16: feeds a bf16 matmul)
                h_sb = setup_sb.tile([B, D], bf16, tag="h")
                nc.gpsimd.dma_start(out=h_sb, in_=h_d[:])
                hT = setup_sb.tile([P, KE, B], bf16, tag="hT")
                for kd in range(KE):
                    ps = setup_ps.tile([P, B], bf16, tag="psb")
                    nc.tensor.matmul(
                        ps, lhsT=h_sb[0:B, kd * P : (kd + 1) * P],
                        rhs=ident_bf16[0:B, 0:B], is_transpose=True,
                    )
                    nc.vector.tensor_copy(out=hT[:, kd, :], in_=ps)

                # W_dec -> wdecT [d_p, kd, a] (bf16, cast during DMA)
                wdec_nat = setup_sb.tile([P, AT, D], bf16, tag="wnat")
                nc.gpsimd.dma_start(
                    out=wdec_nat, in_=wdec_d[:].rearrange("(at p) d -> p at d", p=P)
                )
                wdecT = setup_sb.tile([P, KE, D], bf16, tag="wdecT")
                for at in range(AT):
                    for kd in range(KE):
                        ps = setup_ps.tile([P, P], bf16, tag="psb")
                        nc.tensor.matmul(
                            ps, lhsT=wdec_nat[:, at, kd * P : (kd + 1) * P],
                            rhs=ident_bf16[:], is_transpose=True,
                        )
                        nc.vector.tensor_copy(
                            out=wdecT[:, kd, at * P : (at + 1) * P], in_=ps
                        )

                # attn1T[a, b] = sum_d Wdec[a, d] h[b, d]
                for at in range(AT):
                    ps = setup_ps.tile([P, B], f32, tag="ps")
                    for kd in range(KE):
                        nc.tensor.matmul(
                            ps, lhsT=wdecT[:, kd, at * P : (at + 1) * P],
                            rhs=hT[:, kd, :],
                            start=(kd == 0), stop=(kd == KE - 1),
                        )
                    nc.vector.tensor_copy(out=attn1T[:, at, :], in_=ps)

                # W_enc -> wencT [e_p, ke, a] (bf16, cast during DMA)
                wenc_nat = setup_sb.tile([P, AT, D], bf16, tag="wnat2")
                nc.gpsimd.dma_start(
                    out=wenc_nat, in_=wenc_d[:].rearrange("(at p) e -> p at e", p=P)
                )
                for at in range(AT):
                    for ke in range(KE):
                        ps = setup_ps.tile([P, P], bf16, tag="psb")
                        nc.tensor.matmul(
                            ps, lhsT=wenc_nat[:, at, ke * P : (ke + 1) * P],
                            rhs=ident_bf16[:], is_transpose=True,
                        )
                        nc.vector.tensor_copy(
                            out=wencT[:, ke, at * P : (at + 1) * P], in_=ps
                        )

                # w_full -> [a_p, at] (bf16)
                wf_nat = setup_sb.tile([1, D], bf16, tag="wf")
                nc.gpsimd.dma_start(out=wf_nat, in_=wf_d[:])
                # bf16 PSUM writes must be 4B-aligned: use stride-2 columns
                ps = setup_ps.tile([P, 2 * AT], bf16, tag="psb")
                for at in range(AT):
                    nc.tensor.matmul(
                        ps[:, 2 * at : 2 * at + 1],
                        lhsT=wf_nat[0:1, at * P : (at + 1) * P],
                        rhs=ident_bf16[0:1, 0:1], is_transpose=True,
                        start=(at == 0), stop=(at == AT - 1),
                    )
                nc.vector.tensor_copy(
                    out=wfull_sb,
                    in_=ps.rearrange("p (t two) -> p t two", two=2)[:, :, 0],
                )

            # ---------------- main per-batch pipeline ----------------
            import contextlib as _ctxlib

            _stack = _ctxlib.ExitStack()
            with _stack:
                if vt_mode in ("pe", "xbar"):
                    ps_vt = _stack.enter_context(
                        tc.tile_pool(name="ps_vt", bufs=2, space="PSUM")
                    )
                ps_a2 = _stack.enter_context(
                    tc.tile_pool(name="ps_a2", bufs=2, space="PSUM")
                )
                ps_sc = _stack.enter_context(
                    tc.tile_pool(name="ps_sc", bufs=2, space="PSUM")
                )
                ps_al = _stack.enter_context(
                    tc.tile_pool(name="ps_al", bufs=1, space="PSUM")
                )
                ps_cx = _stack.enter_context(
                    tc.tile_pool(name="ps_cx", bufs=1, space="PSUM")
                )
                import contextlib

                loop_cm = (
                    tc.For_i(0, loop_iters, 1)
                    if loop_iters is not None
                    else contextlib.nullcontext()
                )
                with loop_cm:
                    _batch_body = None  # noqa (marker)
                    for b in [bi for _ in range(reps) for bi in range(nb)]:
                        # load V[b] as bf16, natural layout [s_p, t, e]
                        v_nat = vpool.tile([P, NT, D], bf16)
                        nc.gpsimd.dma_start(
                            out=v_nat,
                            in_=v_d[b].rearrange("(t p) e -> p t e", p=P),
                        )

                        # transpose to vt [e_p, ke, s]
                        if vt_mode == "xbar4":
                            # interleaved layout vt2[pe, t*KE+ke, sl]
                            vt2 = vtpool.tile([P, NT * KE, P], bf16, tag="vt")
                            tg = NT // xbar_calls  # t-tiles per call
                            for g in range(xbar_calls):
                                nc.sync.dma_start_transpose(
                                    out=vt2[:, g * tg * KE : (g + 1) * tg * KE, :],
                                    in_=v_nat[:, g * tg : (g + 1) * tg, :],
                                )
                            vt = None
                            vt2_r = vt2.rearrange("p (t k) s -> p t k s", k=KE)
                        else:
                            vt2_r = None
                            vt = vtpool.tile([P, KE, SEQ], bf16, tag="vt")
                        if vt_mode == "xbar" and not skip_vt:
                            for t in range(NT):
                                nc.sync.dma_start_transpose(
                                    out=vt[:, :, t * P : (t + 1) * P],
                                    in_=v_nat[:, t, :],
                                )
                        elif vt_mode == "xbar4":
                            pass
                        elif not skip_vt:
                            for ke in range(KE):
                                for tg in range(NT // 4):
                                    pvt = ps_vt.tile([P, 512], bf16)
                                    for j in range(4):
                                        t = tg * 4 + j
                                        nc.tensor.matmul(
                                            pvt[:, j * P : (j + 1) * P],
                                            lhsT=v_nat[:, t, ke * P : (ke + 1) * P],
                                            rhs=ident_bf16[:], is_transpose=True,
                                            start=(j == 0), stop=(j == 3),
                                        )
                                    nc.vector.tensor_copy(
                                        out=vt[:, ke, tg * 512 : (tg + 1) * 512], in_=pvt
                                    )
                        else:
                            nc.vector.memset(vt[:, 0, 0:2], 0.5)

                        exp_sb = smpool.tile([1, SEQ], bf16, tag="exp")
                        sums_sb = smpool.tile([1, SC], f32, tag="sums")

                        for sp in range(SC // 2):
                            # two s-chunks per pass: [128,1024] PSUM + one tanh
                            th_tiles = []
                            for at in range(AT):
                                pa2 = ps_a2.tile([P, 1024], f32)
                                for half in range(2):
                                    sc = 2 * sp + half
                                    dst = pa2[:, half * 512 : (half + 1) * 512]
                                    for ke in range(ke_count):
                                        if vt2_r is not None:
                                            rhs = vt2_r[:, 4 * sc : 4 * sc + 4, ke, :]
                                        else:
                                            rhs = vt[:, ke, sc * 512 : (sc + 1) * 512]
                                        nc.tensor.matmul(
                                            dst,
                                            lhsT=wencT[:, ke, at * P : (at + 1) * P],
                                            rhs=rhs,
                                            start=(ke == 0), stop=(ke == ke_count - 1),
                                        )
                                th = tanhpool.tile([P, 1024], bf16)
                                nc.scalar.activation(
                                    out=th, in_=pa2, func=Tanh,
                                    bias=attn1T[:, at, b : b + 1], scale=1.0,
                                )
                                th_tiles.append(th)
                            for half in range(2):
                                sc = 2 * sp + half
                                psc = ps_sc.tile([1, 512], f32)
                                n_sc_mm = 1 if skip_scores else AT
                                for at in range(n_sc_mm):
                                    nc.tensor.matmul(
                                        psc, lhsT=wfull_sb[:, at : at + 1],
                                        rhs=th_tiles[at][:, half * 512 : (half + 1) * 512],
                                        start=(at == 0), stop=(at == n_sc_mm - 1),
                                    )
                                nc.scalar.activation(
                                    out=exp_sb[0:1, sc * 512 : (sc + 1) * 512],
                                    in_=psc, func=Exp,
                                    accum_out=sums_sb[0:1, sc : sc + 1],
                                )

                        # alpha = exp scores scattered down partitions: [s_p, t]
                        # bf16 PSUM writes must be 4B-aligned: use stride-2 columns
                        pal = ps_al.tile([P, 2 * NT], bf16)
                        for t in range(NT):
                            nc.tensor.matmul(
                                pal[:, 2 * t : 2 * t + 1],
                                lhsT=exp_sb[0:1, t * P : (t + 1) * P],
                                rhs=ident_bf16[0:1, 0:1], is_transpose=True,
                                start=(t == 0), stop=(t == NT - 1),
                            )
                        alpha_sb = smpool.tile(
                            [P, NT], f32 if ctx_mode == "dve" else bf16, tag="alpha"
                        )
                        nc.vector.tensor_copy(
                            out=alpha_sb,
                            in_=pal.rearrange("p (t two) -> p t two", two=2)[:, :, 0],
                        )

                        sumtot = smpool.tile([1, 1], f32, tag="sumtot")
                        nc.vector.tensor_reduce(
                            out=sumtot, in_=sums_sb, axis=X, op=ADD
                        )
                        recip = smpool.tile([1, 1], f32, tag="recip")
                        nc.vector.reciprocal(out=recip, in_=sumtot)

                        pcx = ps_cx.tile([1, D], f32)
                        if ctx_mode == "dve":
                            acc = smpool.tile([P, D], f32, tag="acc")
                            nc.vector.tensor_scalar_mul(
                                out=acc, in0=v_nat[:, 0, :],
                                scalar1=alpha_sb[:, 0:1],
                            )
                            for t in range(1, NT):
                                nc.vector.scalar_tensor_tensor(
                                    out=acc, in0=v_nat[:, t, :],
                                    scalar=alpha_sb[:, t : t + 1], in1=acc,
                                    op0=mybir.AluOpType.mult,
                                    op1=mybir.AluOpType.add,
                                )
                            nc.tensor.matmul(pcx, lhsT=ones_f32, rhs=acc)
                        else:
                            for t in range(NT):
                                nc.tensor.matmul(
                                    pcx, lhsT=alpha_sb[:, t : t + 1],
                                    rhs=v_nat[:, t, :],
                                    start=(t == 0), stop=(t == NT - 1),
                                )
                        ctx_b = smpool.tile([1, D], f32, tag="ctx")
                        nc.vector.tensor_scalar_mul(out=ctx_b, in0=pcx, scalar1=recip)
                        nc.sync.dma_start(out=out_d[b], in_=ctx_b)

    _split_waits(nc)
    return nc


def kernel(h, V, W_dec, W_enc, w_full):
    from concourse.bass_utils import run_bass_kernel_spmd

    nc = _CACHE.get("nc")
    if nc is None:
        nc = _CACHE["nc"] = _build()

    h = np.ascontiguousarray(h, dtype=np.float32)
    V = np.ascontiguousarray(V, dtype=np.float32)
    in_maps = []
    for c in range(N_CORES):
        sl = slice(c * B, (c + 1) * B)
        in_maps.append(
            {
                "h": h[sl],
                "V": V[sl],
                "W_dec": np.ascontiguousarray(W_dec, dtype=np.float32),
                "W_enc": np.ascontiguousarray(W_enc, dtype=np.float32),
                "w_full": np.ascontiguousarray(w_full, dtype=np.float32),
            }
        )
    res = run_bass_kernel_spmd(nc, in_maps, core_ids=list(range(N_CORES)))
    out = np.concatenate([res.results[c]["out"] for c in range(N_CORES)], axis=0)
    return out.astype(np.float32)

